# revision 27
# baseline (speedup 1.0000x reference)
"""Trainium2 Bass kernel for nn_AttentionBlock (B=1, S=2048, D=2048, H=32, Dh=64).

Tensor-parallel over heads across 8 NeuronCores (4 heads/core):
  - QKV projection from host-pre-transposed x^T (fp32r/TF32): q^T,k^T
    channel-major, v sequence-major -- one sweep over x, exactly 8 PSUM banks.
  - RoPE: rotate-half term via a signed-permutation PE matmul (rot = P_rot @ q),
    then 3 DVE tensor ops per tile; scale 1/sqrt(dh) folded into Wq on host.
  - Scores computed transposed S^T[k, q] in fp32r; softmax max-subtraction is
    skipped (scores are bounded ~|6| for this model scale), exp on ACT with the
    causal triangle masked post-exp via a 0/1 multiply; P^T stored bf16.
  - PV in bf16 with ones-augmented V (M=65) -> the softmax denominator rides in
    PSUM row 64 for free; normalize via DVE reciprocal + GpSimd
    partition_broadcast + DVE multiply.
  - Two AllGathers (bf16, 512 KB/rank each) fire per head-pair so o_proj
    overlaps the second half of attention; o_proj is column-sliced per core
    (bf16), gated residual applied in fp32, host concatenates the 8 slices.
"""

import numpy as np
import ml_dtypes

import concourse.bacc as bacc
import concourse.mybir as mybir
import concourse.tile as tile
from concourse.bass_utils import run_bass_kernel_spmd
from neuronxcc.starfish.support.dtype import static_cast_fp32_to_fp32r

F32 = mybir.dt.float32
F32R = mybir.dt.float32r
BF16 = mybir.dt.bfloat16
AF = mybir.ActivationFunctionType

S = 2048
D = 2048
H = 32
DH = 64
NCORES = 8
HC = H // NCORES          # 4 heads per core
E = HC * DH               # 256 channels per core
ROPE_BASE = 10000.0
NK = D // 128             # 16 contraction tiles
NP = 4                    # s-quarter passes in stage Q
QB = S // 4               # 512 q-block
BF_NP = np.dtype(ml_dtypes.bfloat16)


def _round_fp32r(x: np.ndarray) -> np.ndarray:
    flat = np.ascontiguousarray(x, dtype=np.float32).ravel().view(np.uint32)
    r = np.asarray(static_cast_fp32_to_fp32r(flat))
    return r.view(np.uint32).astype(np.uint32).view(np.float32).reshape(x.shape)


def _emit_body(nc, t_in, rep, stages="QAO"):
    """Emit one full forward pass. t_in: dict of DRAM APs. rep: suffix for names."""
    xt = t_in["xt"]; wqkt = t_in["wqkt"]; wvt = t_in["wvt"]; wot = t_in["wot"]
    prott = t_in["prott"]; cost = t_in["cost"]; sint = t_in["sint"]
    tri = t_in["tri"]; gatec = t_in["gatec"]; xres = t_in["xres"]
    agin = t_in["agin"]; outc = t_in["outc"]
    agout_a = t_in["agout_a"]; agout_b = t_in["agout_b"]

    with (
        tile.TileContext(nc) as tc,
        tc.tile_pool(name=f"sb{rep}", bufs=1) as sb,
        tc.tile_pool(name=f"sbs{rep}", bufs=3) as sbs,
    ):
        sbq_cm = tc.tile_pool(name=f"sbq{rep}", bufs=1)
        sbq = sbq_cm.__enter__()
        # ---------------- resident constants / weights -----------------
        wqk_all = sb.tile([128, NK * 512], F32R, tag="wqk", name="wqk")
        for k in range(NK):
            nc.sync.dma_start(out=wqk_all[:, k * 512:(k + 1) * 512],
                              in_=wqkt[k * 128:(k + 1) * 128, :])
        wv_all = sb.tile([128, NK * 256], F32R, tag="wv", name="wv")
        for k in range(NK):
            nc.sync.dma_start(out=wv_all[:, k * 256:(k + 1) * 256],
                              in_=wvt[k * 128:(k + 1) * 128, :])
        wo_all = sb.tile([128, NK * 256], BF16, tag="wo", name="wo")
        for k in range(NK):
            nc.sync.dma_start(out=wo_all[:, k * 256:(k + 1) * 256],
                              in_=wot[k * 128:(k + 1) * 128, :])
        prott_s = sb.tile([128, 128], F32R, tag="prott", name="prott")
        nc.sync.dma_start(out=prott_s[:], in_=prott)
        cos_s = sbq.tile([128, S], F32, tag="cos", name="cos")
        nc.sync.dma_start(out=cos_s[:], in_=cost)
        sin_s = sbq.tile([128, S], F32, tag="sin", name="sin")
        nc.sync.dma_start(out=sin_s[:], in_=sint)
        tri_s = sb.tile([128, 128], BF16, tag="tri", name="tri")
        nc.sync.dma_start(out=tri_s[:], in_=tri)
        gate_s = sb.tile([128, 2], F32, tag="gate", name="gate")
        nc.sync.dma_start(out=gate_s[:], in_=gatec.rearrange("(b a) c -> a (b c)", b=2))

        # roped q^T,k^T: 4 tiles of (128, S) fp32r; v': 16 tiles (128, 260) bf16
        qkt_all = sb.tile([128, 4 * S], F32R, tag="qkt", name="qkt")
        v_all = sb.tile([128, 16 * 260], BF16, tag="vall", name="vall")
        for j in range(16):
            nc.vector.memset(v_all[:, j * 260:(j + 1) * 260]
                             .rearrange("p (h w) -> p h w", h=4, w=65)[:, :, 64:65], 1.0)

        # ---------------- stage Q: QKV + RoPE ---------------------------
        ppq_cm = tc.tile_pool(name=f"ppq{rep}", bufs=1, space="PSUM")
        ppt = ppq_cm.__enter__()
        for p in range(NP):
            sc = slice(p * 512, (p + 1) * 512)
            qk_ps = [ppt.tile([128, 512], F32, tag=f"bank{m}", name=f"qkps{p}_{m}")
                     for m in range(4)]
            v_ps = [ppt.tile([128, 256], F32, tag=f"bank{4 + j}", name=f"vps{p}_{j}")
                    for j in range(4)]
            for k in range(NK):
                xq = sbq.tile([128, 512], F32R, tag="xq", bufs=8, name=f"xq{p}_{k}")
                nc.sync.dma_start(out=xq[:], in_=xt[k * 128:(k + 1) * 128, sc])
                for m in range(4):
                    nc.tensor.matmul(qk_ps[m][:],
                                     wqk_all[:, k * 512 + m * 128: k * 512 + (m + 1) * 128],
                                     xq[:], start=(k == 0), stop=(k == NK - 1))
                for j in range(4):
                    nc.tensor.matmul(v_ps[j][:], xq[:, j * 128:(j + 1) * 128],
                                     wv_all[:, k * 256:(k + 1) * 256],
                                     start=(k == 0), stop=(k == NK - 1))
            # RoPE on the 4 qk m-tiles of this pass
            for m in range(4):
                u = sbq.tile([128, 512], F32R, tag="u", bufs=3, name=f"u{p}_{m}")
                nc.vector.tensor_mul(u[:], qk_ps[m][:], sin_s[:, sc])
                t1 = sbq.tile([128, 512], F32, tag="t1", bufs=3, name=f"t1{p}_{m}")
                nc.vector.tensor_mul(t1[:], qk_ps[m][:], cos_s[:, sc])
                rot_ps = ppt.tile([128, 512], F32, tag=f"bank{4 + m}", name=f"rot{p}_{m}")
                nc.tensor.matmul(rot_ps[:], prott_s[:], u[:], start=True, stop=True)
                nc.vector.tensor_add(
                    qkt_all[:, m * S + p * 512: m * S + (p + 1) * 512],
                    t1[:], rot_ps[:])
            # v' evacuation (strided into [vA|1|vB|1|vC|1|vD|1] layout)
            for j in range(4):
                st = p * 4 + j
                dst = v_all[:, st * 260:(st + 1) * 260] \
                    .rearrange("p (h w) -> p h w", h=4, w=65)[:, :, 0:64]
                nc.scalar.copy(
                    dst, v_ps[j][:].rearrange("p (h w) -> p h w", h=4, w=64))
        ppq_cm.__exit__(None, None, None)
        sbq_cm.__exit__(None, None, None)

        # ---------------- stage A: attention ----------------------------
        if "A" not in stages:
            return
        sba_cm = tc.tile_pool(name=f"sba{rep}", bufs=1)
        sba = sba_cm.__enter__()

        def emit_oproj(ppx):
            # o_proj in 2 sweeps of 4 accumulation groups, reusing the
            # attention pool's psum tags so it can start before attention
            # fully drains.
            korder = [2 * r_ for r_ in range(8)] + [2 * r_ + 1 for r_ in range(8)]
            otags = ["s0", "s1", "pv0", "pv1"]
            for sweep in range(2):
                ccs = (0, 1) if sweep == 0 else (2, 3)
                o_ps = {}
                for jj in range(2):
                    for ci, cc in enumerate(ccs):
                        o_ps[(jj, cc)] = ppx.tile(
                            [128, 512], F32, tag=otags[jj * 2 + ci],
                            name=f"ops{jj}_{cc}")
                for idx, k in enumerate(korder):
                    r_, hh = divmod(k, 2)
                    ag = agout_a if hh == 0 else agout_b
                    for cc in ccs:
                        atc = sba.tile([128, 512], BF16, tag="at", bufs=16,
                                       name=f"at{k}_{cc}")
                        nc.sync.dma_start(
                            out=atc[:],
                            in_=ag[r_ * 128:(r_ + 1) * 128, cc * 512:(cc + 1) * 512])
                        for jj in range(2):
                            nc.tensor.matmul(
                                o_ps[(jj, cc)][:],
                                wo_all[:, k * 256 + jj * 128: k * 256 + (jj + 1) * 128],
                                atc[:], start=(idx == 0), stop=(idx == NK - 1))
                for jj in range(2):
                    for cc in ccs:
                        fin = sba.tile([128, 512], F32, tag="fin", bufs=4,
                                       name=f"fin{jj}_{cc}")
                        xr = sba.tile([128, 512], F32, tag="xr", bufs=4,
                                      name=f"xr{jj}_{cc}")
                        nc.sync.dma_start(
                            out=xr[:], in_=xres[jj * 128:(jj + 1) * 128,
                                                cc * 512:(cc + 1) * 512])
                        nc.vector.scalar_tensor_tensor(
                            out=fin[:], in0=o_ps[(jj, cc)][:],
                            scalar=gate_s[:, jj:jj + 1], in1=xr[:],
                            op0=mybir.AluOpType.mult, op1=mybir.AluOpType.add)
                        nc.sync.dma_start(
                            out=outc[jj * 128:(jj + 1) * 128, cc * 512:(cc + 1) * 512],
                            in_=fin[:])

        with tc.tile_pool(name=f"ppa{rep}", bufs=1, space="PSUM") as ppa:
            QW = 1024
            for h in range(4):
                hm, hr = divmod(h, 2)
                qrows = slice(hr * 64, (hr + 1) * 64)
                for J in range(2):
                    nt = 8 * J + 8
                    pv = ppa.tile([65, QW], F32, tag=f"pv{J % 2}", name=f"pv{h}_{J}")
                    for t in range(nt):
                        off = max(0, 128 * t - QW * J)
                        w = QW - off
                        s_ps = ppa.tile([128, QW], F32, tag=f"s{t % 2}",
                                        name=f"s{h}_{J}_{t}")
                        kt = qkt_all[:, (2 + hm) * S + t * 128:(2 + hm) * S + (t + 1) * 128]
                        qt = qkt_all[:, hm * S + QW * J + off: hm * S + QW * (J + 1)]
                        for half in range(2):
                            lo = half * 512
                            hi = min(w, lo + 512)
                            if hi <= lo:
                                continue
                            nc.tensor.matmul(s_ps[:, lo:hi], kt[qrows, :],
                                             qt[qrows, lo:hi], start=True, stop=True)
                        ptile = sba.tile([128, QW], BF16, tag="pt", bufs=7,
                                         name=f"pt{h}_{J}_{t}")
                        nc.scalar.activation(ptile[:, 0:w], s_ps[:, 0:w], AF.Exp)
                        if off > 0 or 128 * t == QW * J:
                            nc.vector.tensor_mul(ptile[:, 0:128], ptile[:, 0:128],
                                                 tri_s[:])
                        for b0 in (0, 512):
                            lo_p = max(off, b0)
                            hi_p = min(QW, b0 + 512)
                            if lo_p >= hi_p:
                                continue
                            nc.tensor.matmul(
                                pv[:, lo_p:hi_p],
                                v_all[:, t * 260 + h * 65: t * 260 + (h + 1) * 65],
                                ptile[:, lo_p - off:hi_p - off], start=(t == 0),
                                stop=(t == nt - 1 and hi_p == QW),
                                skip_group_check=True)
                    r_sb = sba.tile([1, QW], F32, tag="rsb", name=f"r{h}_{J}")
                    nc.vector.reciprocal(r_sb[:], pv[64:65, :])
                    rb_sb = sba.tile([64, QW], F32, tag="rbsb", name=f"rb{h}_{J}")
                    nc.gpsimd.partition_broadcast(rb_sb[:], r_sb[:])
                    nrm = sba.tile([64, QW], BF16, tag="nrm", name=f"nrm{h}_{J}")
                    nc.vector.tensor_mul(nrm[:], pv[0:64, :], rb_sb[:])
                    nc.sync.dma_start(
                        out=agin[h * 64:(h + 1) * 64, QW * J:QW * (J + 1)],
                        in_=nrm[:])
                if h in (1, 3) and "G" in stages and "O" in stages:
                    nc.gpsimd.collective_compute(
                        "AllGather", mybir.AluOpType.bypass,
                        replica_groups=[list(range(NCORES))],
                        ins=[agin[(h - 1) * 64:(h + 1) * 64, :]],
                        outs=[(agout_a if h == 1 else agout_b)[:]],
                    )
            if "O" in stages and "P" not in stages:
                emit_oproj(ppa)

        if "O" in stages and "P" in stages:
            _emit_oproj_old(nc, tc, sba, rep, wo_all, gate_s, xres, outc,
                            agout_a, agout_b)
        sba_cm.__exit__(None, None, None)

        if "D" in stages:
            dqkt = t_in["dqkt"]; dv = t_in["dv"]
            for m in range(4):
                for q4 in range(4):
                    tmp = sba.tile([128, 512], F32, tag="fin", bufs=4, name=f"dqk{m}_{q4}")
                    nc.vector.tensor_copy(
                        tmp[:], qkt_all[:, m * S + q4 * 512: m * S + (q4 + 1) * 512].bitcast(F32))
                    nc.sync.dma_start(out=dqkt[m * 128:(m + 1) * 128, q4 * 512:(q4 + 1) * 512],
                                      in_=tmp[:])
            for j in range(16):
                tmpv = sba.tile([128, 260], F32, tag="dbgv", bufs=1, name=f"dv{j}")
                nc.vector.tensor_copy(tmpv[:], v_all[:, j * 260:(j + 1) * 260])
                nc.sync.dma_start(out=dv[j * 128:(j + 1) * 128, :], in_=tmpv[:])

        if "O" not in stages:
            sba_cm.__exit__(None, None, None)
            return

        if "D" in stages:
            for kk in range(NK):
                r_, hh = divmod(kk, 2)
                ag = agout_a if hh == 0 else agout_b
                for cc2 in range(4):
                    tmpa = sba.tile([128, 512], F32, tag="fin", bufs=4,
                                    name=f"da{kk}_{cc2}")
                    src_ap = ag[r_ * 128:(r_ + 1) * 128, cc2 * 512:(cc2 + 1) * 512]
                    nc.gpsimd.dma_start(out=tmpa[:], in_=src_ap)
                    nc.sync.dma_start(
                        out=t_in["dag"][kk * 128:(kk + 1) * 128, cc2 * 512:(cc2 + 1) * 512],
                        in_=tmpa[:])



def _emit_oproj_old(nc, tc, sbs, rep, wo_all, gate_s, xres, outc, agout_a, agout_b):
    ppo_cm = tc.tile_pool(name=f"ppo{rep}", bufs=1, space="PSUM")
    ppo = ppo_cm.__enter__()
    o_ps = [ppo.tile([128, 512], F32, tag=f"bank{4 * jj + cc}",
                     name=f"ops{jj}_{cc}")
            for jj in range(2) for cc in range(4)]
    korder = [2 * r for r in range(8)] + [2 * r + 1 for r in range(8)]
    for idx, k in enumerate(korder):
        r_, hh = divmod(k, 2)
        ag = agout_a if hh == 0 else agout_b
        for cc in range(4):
            atc = sba.tile([128, 512], BF16, tag="at", bufs=16, name=f"at{k}_{cc}")
            nc.sync.dma_start(
                out=atc[:], in_=ag[r_ * 128:(r_ + 1) * 128, cc * 512:(cc + 1) * 512])
            for jj in range(2):
                nc.tensor.matmul(
                    o_ps[jj * 4 + cc][:],
                    wo_all[:, k * 256 + jj * 128: k * 256 + (jj + 1) * 128],
                    atc[:], start=(idx == 0), stop=(idx == NK - 1))
    for jj in range(2):
        for cc in range(4):
            fin = sba.tile([128, 512], F32, tag="fin", bufs=4, name=f"fin{jj}_{cc}")
            xr = sba.tile([128, 512], F32, tag="xr", bufs=4, name=f"xr{jj}_{cc}")
            nc.sync.dma_start(
                out=xr[:], in_=xres[jj * 128:(jj + 1) * 128, cc * 512:(cc + 1) * 512])
            nc.vector.scalar_tensor_tensor(
                out=fin[:], in0=o_ps[jj * 4 + cc][:],
                scalar=gate_s[:, jj:jj + 1], in1=xr[:],
                op0=mybir.AluOpType.mult, op1=mybir.AluOpType.add)
            nc.sync.dma_start(
                out=outc[jj * 128:(jj + 1) * 128, cc * 512:(cc + 1) * 512],
                in_=fin[:])
    ppo_cm.__exit__(None, None, None)


def build_nc(reps: int = 1, stages: str = "QAO"):
    nc = bacc.Bacc("TRN2", target_bir_lowering=False, debug=False,
                   num_devices=NCORES)
    t_in = {
        "xt": nc.dram_tensor("xt", [D, S], F32R, kind="ExternalInput").ap(),
        "wqkt": nc.dram_tensor("wqkt", [D, 512], F32R, kind="ExternalInput").ap(),
        "wvt": nc.dram_tensor("wvt", [D, E], F32R, kind="ExternalInput").ap(),
        "wot": nc.dram_tensor("wot", [D, E], BF16, kind="ExternalInput").ap(),
        "prott": nc.dram_tensor("prott", [128, 128], F32R, kind="ExternalInput").ap(),
        "cost": nc.dram_tensor("cost", [128, S], F32, kind="ExternalInput").ap(),
        "sint": nc.dram_tensor("sint", [128, S], F32, kind="ExternalInput").ap(),
        "tri": nc.dram_tensor("tri", [128, 128], BF16, kind="ExternalInput").ap(),
        "gatec": nc.dram_tensor("gatec", [E, 1], F32, kind="ExternalInput").ap(),
        "xres": nc.dram_tensor("xres", [E, S], F32, kind="ExternalInput").ap(),
        "outc": nc.dram_tensor("outc", [E, S], F32, kind="ExternalOutput").ap(),
    }
    if "D" in stages:
        t_in["dqkt"] = nc.dram_tensor("dqkt", [512, S], F32, kind="ExternalOutput").ap()
        t_in["dv"] = nc.dram_tensor("dv", [16 * 128, 260], F32, kind="ExternalOutput").ap()
        t_in["dag"] = nc.dram_tensor("dag", [D, S], F32, kind="ExternalOutput").ap()
    for r in range(reps):
        t_in["agin"] = nc.dram_tensor(f"agin{r}", [E, S], BF16).ap()
        t_in["agout_a"] = nc.dram_tensor(f"agouta{r}", [NCORES * 128, S], BF16,
                                         addr_space="Shared").ap()
        t_in["agout_b"] = nc.dram_tensor(f"agoutb{r}", [NCORES * 128, S], BF16,
                                         addr_space="Shared").ap()
        _emit_body(nc, t_in, r, stages)
    nc.compile()
    return nc


def prep_inputs(x, Wqkv, Wo, gate):
    """Host-side sharding/layout prep. Returns in_maps for 8 cores."""
    x2 = np.ascontiguousarray(np.asarray(x, dtype=np.float32).reshape(S, D))
    Wqkv = np.asarray(Wqkv, dtype=np.float32)
    Wo = np.asarray(Wo, dtype=np.float32)
    gate = np.asarray(gate, dtype=np.float32)

    xt_r = _round_fp32r(x2.T)                               # (D, S)

    # RoPE tables (match reference: float32 math)
    inv_freq = (1.0 / (ROPE_BASE **
                       (np.arange(0, DH, 2, dtype=np.float32) / DH))).astype(np.float32)
    freqs = np.arange(S, dtype=np.float32)[:, None] * inv_freq[None, :]
    emb = np.concatenate([freqs, freqs], axis=-1)           # (S, DH)
    cos = np.cos(emb).astype(np.float32)                    # (S, 64)
    sin = np.sin(emb).astype(np.float32)
    cost = np.ascontiguousarray(np.vstack([cos.T, cos.T]))  # (128, S)
    sint = np.ascontiguousarray(np.vstack([sin.T, sin.T]))

    prot = np.zeros((64, 64), np.float32)
    for i in range(32):
        prot[i, i + 32] = -1.0
        prot[i + 32, i] = 1.0
    prott = np.zeros((128, 128), np.float32)
    prott[0:64, 0:64] = prot.T
    prott[64:128, 64:128] = prot.T
    prott = _round_fp32r(prott)

    kk = np.arange(128)[:, None]
    qq = np.arange(128)[None, :]
    tri = (kk <= qq).astype(BF_NP)                          # (128,128) 0/1

    scale = np.float32(1.0 / np.sqrt(DH))
    in_maps = []
    for c in range(NCORES):
        rs = slice(c * E, (c + 1) * E)
        wq = Wqkv[0 * D + c * E: 0 * D + (c + 1) * E, :] * scale   # (256, D)
        wk = Wqkv[1 * D + c * E: 1 * D + (c + 1) * E, :]
        wv = Wqkv[2 * D + c * E: 2 * D + (c + 1) * E, :]
        wqkt = _round_fp32r(np.vstack([wq, wk]).T)          # (D, 512)
        wvt = _round_fp32r(wv.T)                            # (D, 256)
        wot = Wo[rs, :].T.astype(BF_NP)                     # (D, 256)
        in_maps.append({
            "xt": xt_r,
            "wqkt": np.ascontiguousarray(wqkt),
            "wvt": np.ascontiguousarray(wvt),
            "wot": np.ascontiguousarray(wot),
            "prott": prott,
            "cost": cost,
            "sint": sint,
            "tri": np.ascontiguousarray(tri),
            "gatec": np.ascontiguousarray(gate[rs, None]),
            "xres": np.ascontiguousarray(x2.T[rs, :]),
        })
    return in_maps


_NC_CACHE = {}


def run(inputs, reps: int = 1, nc=None):
    if nc is None:
        if reps not in _NC_CACHE:
            _NC_CACHE[reps] = build_nc(reps, stages="QAOG")
        nc = _NC_CACHE[reps]
    in_maps = prep_inputs(inputs["x"], inputs["Wqkv"], inputs["Wo"], inputs["gate"])
    res = run_bass_kernel_spmd(nc, in_maps, core_ids=list(range(NCORES)))
    outT = np.empty((D, S), dtype=np.float32)
    for c in range(NCORES):
        outT[c * E:(c + 1) * E, :] = res.results[c]["outc"]
    return np.ascontiguousarray(outT.T).reshape(1, S, D)


def kernel(**inputs) -> np.ndarray:
    return run(inputs)

